# revision 6
# baseline (speedup 1.0000x reference)
"""Trainium2 Bass kernel for nn_MoE_66803921322559.

Top-2-of-16 MoE (T=2048 tokens, D=1024, INTER=512) + shared expert
(SHARED_INTER=1024), expert-parallel over 8 NeuronCores:

  - core c owns experts (2c, 2c+1); gate computed on-device in fp32
    (selection on logits; sigmoid/normalize for combine weights)
  - routed experts + shared-expert slice computed in bf16 on device
  - partial outputs y [D, T] summed across cores with an on-device
    ReduceScatter; host concatenates the 8 [128, T] shards.
"""

import os
import sys
import types

import numpy as np

sys.path.insert(0, "/opt/trn_rl_repo")

import ml_dtypes

BF = ml_dtypes.bfloat16

B, S, DIM = 2, 1024, 1024
E, K, INTER = 16, 2, 512
T = B * S
N_CORES = 8
EPC = E // N_CORES          # experts per core
SIC = 2 * INTER // N_CORES  # shared-inter slice per core (128)

KD = DIM // 128             # 8 contraction chunks over D
NT = T // 512               # 4 token chunks of 512
MI = INTER // 128           # 4 inter chunks per expert


def _install_ntff_hook():
    """Provide antenv.axon_hooks (missing in this container) so
    run_bass_kernel_spmd(trace=True) can capture NTFF profiles via axon."""
    try:
        import antenv
        if hasattr(antenv, "axon_hooks"):
            return
        from trn_agent_boot.trn_boot import _ntff_profile_via_ctypes
        hook = _ntff_profile_via_ctypes("/opt/axon/libaxon_pjrt.so")
        mod = types.ModuleType("antenv.axon_hooks")
        mod._hook = hook
        mod.get_axon_ntff_profile_hook = lambda: mod._hook
        mod.set_axon_ntff_profile_hook = lambda h: setattr(mod, "_hook", h)
        sys.modules["antenv.axon_hooks"] = mod
        antenv.axon_hooks = mod
    except Exception:
        pass


_install_ntff_hook()

from concourse import bacc, bass, mybir, tile  # noqa: E402
from concourse.bass_utils import run_bass_kernel_spmd  # noqa: E402
from concourse.masks import make_identity  # noqa: E402

F32 = mybir.dt.float32
BF16 = mybir.dt.bfloat16
AF = mybir.ActivationFunctionType
ALU = mybir.AluOpType

last_exec_time_ns = None
_cached = {}


def _build():
    nc = bacc.Bacc("TRN2", target_bir_lowering=False, debug=False,
                   num_devices=N_CORES)

    xtf_d = nc.dram_tensor("xtf", [DIM, T], F32, kind="ExternalInput").ap()
    xtb_d = nc.dram_tensor("xtb", [DIM, T], BF16, kind="ExternalInput").ap()
    gwt_d = nc.dram_tensor("gwt", [DIM, E], F32, kind="ExternalInput").ap()
    w1t_d = nc.dram_tensor("w1t", [EPC, DIM, INTER], BF16, kind="ExternalInput").ap()
    w3t_d = nc.dram_tensor("w3t", [EPC, DIM, INTER], BF16, kind="ExternalInput").ap()
    w2t_d = nc.dram_tensor("w2t", [EPC, INTER, DIM], BF16, kind="ExternalInput").ap()
    sw1t_d = nc.dram_tensor("sw1t", [DIM, SIC], BF16, kind="ExternalInput").ap()
    sw3t_d = nc.dram_tensor("sw3t", [DIM, SIC], BF16, kind="ExternalInput").ap()
    sw2t_d = nc.dram_tensor("sw2t", [SIC, DIM], BF16, kind="ExternalInput").ap()
    out_d = nc.dram_tensor("out", [DIM // N_CORES, T], BF16,
                           kind="ExternalOutput").ap()

    with tile.TileContext(nc) as tc:
        with (
            tc.tile_pool(name="wpool", bufs=1) as wp,
            tc.tile_pool(name="xf", bufs=3) as xfp,
            tc.tile_pool(name="work", bufs=3) as wk,
            tc.tile_pool(name="psum", bufs=2, space="PSUM") as pp,
            tc.tile_pool(name="psc", bufs=2, space="PSUM") as pscp,
            tc.tile_pool(name="dram", bufs=1, space="DRAM") as dp,
        ):
            # ---- persistent SBUF loads -------------------------------------
            xtb = []
            for k in range(KD):
                t_ = wp.tile([128, T], BF16, tag=f"xtb{k}")
                nc.sync.dma_start(out=t_[:], in_=xtb_d[k * 128:(k + 1) * 128, :])
                xtb.append(t_)
            gwt = []
            for k in range(KD):
                t_ = wp.tile([128, E], F32, tag=f"gwt{k}")
                nc.sync.dma_start(out=t_[:], in_=gwt_d[k * 128:(k + 1) * 128, :])
                gwt.append(t_)
            w1s, w3s, w2s = [], [], []
            for e in range(EPC):
                w1s.append([])
                w3s.append([])
                w2s.append([])
                for k in range(KD):
                    t1_ = wp.tile([128, INTER], BF16, tag=f"w1s{e}_{k}")
                    nc.sync.dma_start(out=t1_[:], in_=w1t_d[e, k * 128:(k + 1) * 128, :])
                    w1s[e].append(t1_)
                    t3_ = wp.tile([128, INTER], BF16, tag=f"w3s{e}_{k}")
                    nc.sync.dma_start(out=t3_[:], in_=w3t_d[e, k * 128:(k + 1) * 128, :])
                    w3s[e].append(t3_)
                for m in range(MI):
                    t2_ = wp.tile([128, DIM], BF16, tag=f"w2s{e}_{m}")
                    nc.sync.dma_start(out=t2_[:], in_=w2t_d[e, m * 128:(m + 1) * 128, :])
                    w2s[e].append(t2_)
            sw1s, sw3s = [], []
            for k in range(KD):
                t_ = wp.tile([128, SIC], BF16, tag=f"sw1s{k}")
                nc.sync.dma_start(out=t_[:], in_=sw1t_d[k * 128:(k + 1) * 128, :])
                sw1s.append(t_)
                t_ = wp.tile([128, SIC], BF16, tag=f"sw3s{k}")
                nc.sync.dma_start(out=t_[:], in_=sw3t_d[k * 128:(k + 1) * 128, :])
                sw3s.append(t_)
            sw2s = wp.tile([128, DIM], BF16, tag="sw2s")
            nc.sync.dma_start(out=sw2s[:], in_=sw2t_d[:, :])

            ident = wp.tile([128, 128], F32, tag="ident")
            make_identity(nc, ident[:])

            # ---- gate: fp32 logits [E, T] ----------------------------------
            logits = wp.tile([E, T], F32, tag="logits")
            for n in range(NT):
                nsl = slice(n * 512, (n + 1) * 512)
                psc = pscp.tile([E, 512], F32, tag="pg")
                for k in range(KD):
                    xf = xfp.tile([128, 512], F32, tag="xf")
                    nc.sync.dma_start(
                        out=xf[:], in_=xtf_d[k * 128:(k + 1) * 128, nsl])
                    nc.tensor.matmul(psc[:], lhsT=gwt[k][:], rhs=xf[:],
                                     start=(k == 0), stop=(k == KD - 1))
                nc.vector.tensor_copy(out=logits[:, nsl], in_=psc[:])

            # transpose logits tiles, top-2 select, combine weights
            wrow = [wp.tile([1, T], BF16, tag=f"wrow{e}", name=f"wrow{e}") for e in range(EPC)]
            for i in range(T // 128):
                isl = slice(i * 128, (i + 1) * 128)
                ptr = pscp.tile([128, E], F32, tag="pg", name="ptr")
                nc.tensor.transpose(out=ptr[:], in_=logits[:, isl],
                                    identity=ident[:E, :E])
                lg = wk.tile([128, E], F32, tag="lg")
                nc.vector.tensor_copy(out=lg[:], in_=ptr[:])
                mx = wk.tile([128, 8], F32, tag="mx")
                nc.vector.max(out=mx[:], in_=lg[:])
                # sigmoid of top-2 logits -> denominator -> reciprocal
                s12 = wk.tile([128, 2], F32, tag="s12")
                nc.scalar.activation(out=s12[:], in_=mx[:, 0:2], func=AF.Sigmoid)
                den = wk.tile([128, 1], F32, tag="den")
                nc.vector.tensor_tensor(out=den[:], in0=s12[:, 0:1],
                                        in1=s12[:, 1:2], op=ALU.add)
                rec = wk.tile([128, 1], F32, tag="rec")
                nc.vector.reciprocal(out=rec[:], in_=den[:])
                # my experts: cols 0..EPC of permuted gate
                sel = wk.tile([128, EPC], F32, tag="sel")
                nc.vector.tensor_tensor(
                    out=sel[:], in0=lg[:, 0:EPC],
                    in1=mx[:, 1:2].to_broadcast([128, EPC]), op=ALU.is_ge)
                sg = wk.tile([128, EPC], F32, tag="sg")
                nc.scalar.activation(out=sg[:], in_=lg[:, 0:EPC], func=AF.Sigmoid)
                wcol = wk.tile([128, EPC], F32, tag="wcol")
                nc.vector.tensor_tensor(out=wcol[:], in0=sg[:], in1=sel[:],
                                        op=ALU.mult)
                nc.vector.tensor_scalar(out=wcol[:], in0=wcol[:],
                                        scalar1=rec[:, 0:1], scalar2=None,
                                        op0=ALU.mult)
                # spread expert cols 32 apart so the transposed rows land on
                # legal partition bases (0, 32), then transpose
                wc64 = wk.tile([128, 32 * EPC], F32, tag="wc64")
                for e in range(EPC):
                    nc.vector.tensor_copy(out=wc64[:, 32 * e:32 * e + 1],
                                          in_=wcol[:, e:e + 1])
                pwt = pscp.tile([32 * EPC, 128], F32, tag="pg", name="pwt")
                nc.tensor.transpose(out=pwt[:], in_=wc64[:], identity=ident[:])
                for e in range(EPC):
                    nc.vector.tensor_copy(out=wrow[e][:, isl],
                                          in_=pwt[32 * e:32 * e + 1, :])

            # broadcast combine weights across partitions: [1,T] -> [128,T]
            wbc = []
            for e in range(EPC):
                t_ = wp.tile([128, T], BF16, tag=f"wbc{e}")
                nc.gpsimd.partition_broadcast(t_[:], wrow[e][:, :])
                wbc.append(t_)

            # ---- routed experts: h = silu(x@w1t) * (x@w3t) * gate ----------
            hsb = [[wp.tile([128, T], BF16, tag=f"hsb{e}_{m}", name=f"hsb{e}_{m}") for m in range(MI)]
                   for e in range(EPC)]
            hss = wp.tile([128, T], BF16, tag="hss")

            for e in range(EPC):
                for m in range(MI):
                    msl = slice(m * 128, (m + 1) * 128)
                    for n in range(NT):
                        nsl = slice(n * 512, (n + 1) * 512)
                        ps1 = pp.tile([128, 512], F32, tag="ps1")
                        ps3 = pp.tile([128, 512], F32, tag="ps3")
                        for k in range(KD):
                            nc.tensor.matmul(ps1[:], lhsT=w1s[e][k][:, msl],
                                             rhs=xtb[k][:, nsl],
                                             start=(k == 0), stop=(k == KD - 1))
                        for k in range(KD):
                            nc.tensor.matmul(ps3[:], lhsT=w3s[e][k][:, msl],
                                             rhs=xtb[k][:, nsl],
                                             start=(k == 0), stop=(k == KD - 1))
                        t1 = wk.tile([128, 512], BF16, tag="t1")
                        nc.scalar.activation(out=t1[:], in_=ps1[:], func=AF.Silu)
                        t3 = wk.tile([128, 512], BF16, tag="t3")
                        nc.vector.tensor_copy(out=t3[:], in_=ps3[:])
                        tp = wk.tile([128, 512], BF16, tag="tp")
                        nc.vector.tensor_tensor(out=tp[:], in0=t1[:], in1=t3[:],
                                                op=ALU.mult)
                        nc.vector.tensor_tensor(out=hsb[e][m][:, nsl], in0=tp[:],
                                                in1=wbc[e][:, nsl], op=ALU.mult)

            # ---- shared expert slice ---------------------------------------
            for n in range(NT):
                nsl = slice(n * 512, (n + 1) * 512)
                ps1 = pp.tile([128, 512], F32, tag="ps1")
                ps3 = pp.tile([128, 512], F32, tag="ps3")
                for k in range(KD):
                    nc.tensor.matmul(ps1[:], lhsT=sw1s[k][:], rhs=xtb[k][:, nsl],
                                     start=(k == 0), stop=(k == KD - 1))
                for k in range(KD):
                    nc.tensor.matmul(ps3[:], lhsT=sw3s[k][:], rhs=xtb[k][:, nsl],
                                     start=(k == 0), stop=(k == KD - 1))
                t1 = wk.tile([128, 512], BF16, tag="t1")
                nc.scalar.activation(out=t1[:], in_=ps1[:], func=AF.Silu)
                t3 = wk.tile([128, 512], BF16, tag="t3")
                nc.vector.tensor_copy(out=t3[:], in_=ps3[:])
                nc.vector.tensor_tensor(out=hss[:, nsl], in0=t1[:], in1=t3[:],
                                        op=ALU.mult)

            # ---- y[d, t] = shared + sum_e h_e @ w2t_e ----------------------
            y_dram = dp.tile([DIM, T], BF16)
            y_rs = dp.tile([DIM // N_CORES, T], BF16)
            for d in range(KD):
                dsl = slice(d * 128, (d + 1) * 128)
                ysb = wk.tile([128, T], BF16, tag="ysb")
                for n in range(NT):
                    nsl = slice(n * 512, (n + 1) * 512)
                    psy = pp.tile([128, 512], F32, tag="psy")
                    nc.tensor.matmul(psy[:], lhsT=sw2s[:, dsl], rhs=hss[:, nsl],
                                     start=True, stop=False)
                    for e in range(EPC):
                        for m in range(MI):
                            nc.tensor.matmul(
                                psy[:], lhsT=w2s[e][m][:, dsl],
                                rhs=hsb[e][m][:, nsl],
                                start=False,
                                stop=(e == EPC - 1 and m == MI - 1))
                    nc.vector.tensor_copy(out=ysb[:, nsl], in_=psy[:])
                nc.sync.dma_start(out=y_dram[dsl, :], in_=ysb[:])

            # ---- ReduceScatter over 8 cores + output -----------------------
            nc.gpsimd.collective_compute(
                "ReduceScatter",
                ALU.add,
                replica_groups=[list(range(N_CORES))],
                ins=[y_dram.opt()],
                outs=[y_rs.opt()],
            )
            nc.gpsimd.dma_start(out=out_d[:], in_=y_rs[:])

    nc.compile()
    return nc


def kernel(x, gate_w, w1, w2, w3, sw1, sw2, sw3):
    global last_exec_time_ns

    xt = np.ascontiguousarray(
        np.asarray(x, np.float32).reshape(T, DIM).T)      # [D, T] fp32
    xtb = xt.astype(BF)
    gate_w = np.asarray(gate_w, np.float32)
    w1 = np.asarray(w1, np.float32)
    w2 = np.asarray(w2, np.float32)
    w3 = np.asarray(w3, np.float32)
    sw1 = np.asarray(sw1, np.float32)
    sw2 = np.asarray(sw2, np.float32)
    sw3 = np.asarray(sw3, np.float32)

    in_maps = []
    for c in range(N_CORES):
        mine = [EPC * c + j for j in range(EPC)]
        perm = mine + [e for e in range(E) if e not in mine]
        gwt = np.ascontiguousarray(gate_w[perm].T)        # [D, E] fp32
        w1t = np.stack([np.ascontiguousarray(w1[e].T) for e in mine]).astype(BF)
        w3t = np.stack([np.ascontiguousarray(w3[e].T) for e in mine]).astype(BF)
        w2t = np.stack([np.ascontiguousarray(w2[e].T) for e in mine]).astype(BF)
        ssl = slice(SIC * c, SIC * (c + 1))
        sw1t = np.ascontiguousarray(sw1[ssl, :].T).astype(BF)  # [D, SIC]
        sw3t = np.ascontiguousarray(sw3[ssl, :].T).astype(BF)
        sw2t = np.ascontiguousarray(sw2[:, ssl].T).astype(BF)  # [SIC, D]
        in_maps.append({
            "xtf": xt, "xtb": xtb, "gwt": gwt,
            "w1t": w1t, "w3t": w3t, "w2t": w2t,
            "sw1t": sw1t, "sw3t": sw3t, "sw2t": sw2t,
        })

    if "nc" not in _cached:
        _cached["nc"] = _build()
    nc = _cached["nc"]

    res = run_bass_kernel_spmd(nc, in_maps, core_ids=list(range(N_CORES)))
    last_exec_time_ns = res.exec_time_ns

    yt = np.concatenate([res.results[c]["out"].astype(np.float32)
                         for c in range(N_CORES)], axis=0)  # [D, T]
    return np.ascontiguousarray(yt.T).reshape(B, S, DIM).astype(np.float32)


# revision 8
# speedup vs baseline: 1.0553x; 1.0553x over previous
"""Trainium2 Bass kernel for nn_MoE_66803921322559.

Top-2-of-16 MoE (T=2048 tokens, D=1024, INTER=512) + shared expert
(SHARED_INTER=1024), expert-parallel over 8 NeuronCores:

  - core c owns experts (2c, 2c+1); gate computed on-device in fp32
    (selection on logits; sigmoid/normalize for combine weights)
  - routed experts + shared-expert slice computed in bf16 on device
  - partial outputs y [D, T] summed across cores with an on-device
    ReduceScatter; host concatenates the 8 [128, T] shards.
"""

import os
import sys
import types

import numpy as np

sys.path.insert(0, "/opt/trn_rl_repo")

import ml_dtypes

BF = ml_dtypes.bfloat16

B, S, DIM = 2, 1024, 1024
E, K, INTER = 16, 2, 512
T = B * S
N_CORES = 8
EPC = E // N_CORES          # experts per core
SIC = 2 * INTER // N_CORES  # shared-inter slice per core (128)

KD = DIM // 128             # 8 contraction chunks over D
NT = T // 512               # 4 token chunks of 512
MI = INTER // 128           # 4 inter chunks per expert


def _install_ntff_hook():
    """Provide antenv.axon_hooks (missing in this container) so
    run_bass_kernel_spmd(trace=True) can capture NTFF profiles via axon."""
    try:
        import antenv
        if hasattr(antenv, "axon_hooks"):
            return
        from trn_agent_boot.trn_boot import _ntff_profile_via_ctypes
        hook = _ntff_profile_via_ctypes("/opt/axon/libaxon_pjrt.so")
        mod = types.ModuleType("antenv.axon_hooks")
        mod._hook = hook
        mod.get_axon_ntff_profile_hook = lambda: mod._hook
        mod.set_axon_ntff_profile_hook = lambda h: setattr(mod, "_hook", h)
        sys.modules["antenv.axon_hooks"] = mod
        antenv.axon_hooks = mod
    except Exception:
        pass


_install_ntff_hook()

from concourse import bacc, bass, mybir, tile  # noqa: E402
from concourse.bass_utils import run_bass_kernel_spmd  # noqa: E402
from concourse.masks import make_identity  # noqa: E402

F32 = mybir.dt.float32
BF16 = mybir.dt.bfloat16
AF = mybir.ActivationFunctionType
ALU = mybir.AluOpType

last_exec_time_ns = None
_cached = {}


def _build():
    nc = bacc.Bacc("TRN2", target_bir_lowering=False, debug=False,
                   num_devices=N_CORES)

    xtf_d = nc.dram_tensor("xtf", [DIM, T], F32, kind="ExternalInput").ap()
    xtb_d = nc.dram_tensor("xtb", [DIM, T], BF16, kind="ExternalInput").ap()
    gwt_d = nc.dram_tensor("gwt", [DIM, E], F32, kind="ExternalInput").ap()
    w1t_d = nc.dram_tensor("w1t", [EPC, DIM, INTER], BF16, kind="ExternalInput").ap()
    w3t_d = nc.dram_tensor("w3t", [EPC, DIM, INTER], BF16, kind="ExternalInput").ap()
    w2t_d = nc.dram_tensor("w2t", [EPC, INTER, DIM], BF16, kind="ExternalInput").ap()
    sw1t_d = nc.dram_tensor("sw1t", [DIM, SIC], BF16, kind="ExternalInput").ap()
    sw3t_d = nc.dram_tensor("sw3t", [DIM, SIC], BF16, kind="ExternalInput").ap()
    sw2t_d = nc.dram_tensor("sw2t", [SIC, DIM], BF16, kind="ExternalInput").ap()
    out_d = nc.dram_tensor("out", [DIM // N_CORES, T], BF16,
                           kind="ExternalOutput").ap()

    with tile.TileContext(nc) as tc:
        with (
            tc.tile_pool(name="wpool", bufs=1) as wp,
            tc.tile_pool(name="xf", bufs=3) as xfp,
            tc.tile_pool(name="work", bufs=3) as wk,
            tc.tile_pool(name="psum", bufs=2, space="PSUM") as pp,
            tc.tile_pool(name="psc", bufs=2, space="PSUM") as pscp,
            tc.tile_pool(name="dram", bufs=1, space="DRAM") as dp,
        ):
            # ---- persistent SBUF loads -------------------------------------
            xtb = []
            for k in range(KD):
                t_ = wp.tile([128, T], BF16, tag=f"xtb{k}")
                nc.scalar.dma_start(out=t_[:], in_=xtb_d[k * 128:(k + 1) * 128, :])
                xtb.append(t_)
            gwt = []
            for k in range(KD):
                t_ = wp.tile([128, E], F32, tag=f"gwt{k}")
                nc.sync.dma_start(out=t_[:], in_=gwt_d[k * 128:(k + 1) * 128, :])
                gwt.append(t_)
            w1s, w3s, w2s = [], [], []
            for e in range(EPC):
                w1s.append([])
                w3s.append([])
                w2s.append([])
                for k in range(KD):
                    t1_ = wp.tile([128, INTER], BF16, tag=f"w1s{e}_{k}")
                    nc.scalar.dma_start(out=t1_[:], in_=w1t_d[e, k * 128:(k + 1) * 128, :])
                    w1s[e].append(t1_)
                    t3_ = wp.tile([128, INTER], BF16, tag=f"w3s{e}_{k}")
                    nc.scalar.dma_start(out=t3_[:], in_=w3t_d[e, k * 128:(k + 1) * 128, :])
                    w3s[e].append(t3_)
                for m in range(MI):
                    t2_ = wp.tile([128, DIM], BF16, tag=f"w2s{e}_{m}")
                    nc.scalar.dma_start(out=t2_[:], in_=w2t_d[e, m * 128:(m + 1) * 128, :])
                    w2s[e].append(t2_)
            sw1s, sw3s = [], []
            for k in range(KD):
                t_ = wp.tile([128, SIC], BF16, tag=f"sw1s{k}")
                nc.scalar.dma_start(out=t_[:], in_=sw1t_d[k * 128:(k + 1) * 128, :])
                sw1s.append(t_)
                t_ = wp.tile([128, SIC], BF16, tag=f"sw3s{k}")
                nc.scalar.dma_start(out=t_[:], in_=sw3t_d[k * 128:(k + 1) * 128, :])
                sw3s.append(t_)
            sw2s = wp.tile([128, DIM], BF16, tag="sw2s")
            nc.scalar.dma_start(out=sw2s[:], in_=sw2t_d[:, :])

            ident = wp.tile([128, 128], F32, tag="ident")
            make_identity(nc, ident[:])

            # ---- gate: fp32 logits [E, T] ----------------------------------
            logits = wp.tile([E, T], F32, tag="logits")
            for n in range(NT):
                nsl = slice(n * 512, (n + 1) * 512)
                psc = pscp.tile([E, 512], F32, tag="pg")
                for k in range(KD):
                    xf = xfp.tile([128, 512], F32, tag="xf")
                    nc.sync.dma_start(
                        out=xf[:], in_=xtf_d[k * 128:(k + 1) * 128, nsl])
                    nc.tensor.matmul(psc[:], lhsT=gwt[k][:], rhs=xf[:],
                                     start=(k == 0), stop=(k == KD - 1))
                nc.vector.tensor_copy(out=logits[:, nsl], in_=psc[:])

            # transpose logits tiles, top-2 select, combine weights
            wrow = [wp.tile([1, T], BF16, tag=f"wrow{e}", name=f"wrow{e}") for e in range(EPC)]
            for i in range(T // 128):
                isl = slice(i * 128, (i + 1) * 128)
                ptr = pscp.tile([128, E], F32, tag="pg", name="ptr")
                nc.tensor.transpose(out=ptr[:], in_=logits[:, isl],
                                    identity=ident[:E, :E])
                lg = wk.tile([128, E], F32, tag="lg")
                nc.vector.tensor_copy(out=lg[:], in_=ptr[:])
                mx = wk.tile([128, 8], F32, tag="mx")
                nc.vector.max(out=mx[:], in_=lg[:])
                # sigmoid of top-2 logits -> denominator -> reciprocal
                s12 = wk.tile([128, 2], F32, tag="s12")
                nc.scalar.activation(out=s12[:], in_=mx[:, 0:2], func=AF.Sigmoid)
                den = wk.tile([128, 1], F32, tag="den")
                nc.vector.tensor_tensor(out=den[:], in0=s12[:, 0:1],
                                        in1=s12[:, 1:2], op=ALU.add)
                rec = wk.tile([128, 1], F32, tag="rec")
                nc.vector.reciprocal(out=rec[:], in_=den[:])
                # my experts: cols 0..EPC of permuted gate
                sel = wk.tile([128, EPC], F32, tag="sel")
                nc.vector.tensor_tensor(
                    out=sel[:], in0=lg[:, 0:EPC],
                    in1=mx[:, 1:2].to_broadcast([128, EPC]), op=ALU.is_ge)
                sg = wk.tile([128, EPC], F32, tag="sg")
                nc.scalar.activation(out=sg[:], in_=lg[:, 0:EPC], func=AF.Sigmoid)
                wcol = wk.tile([128, EPC], F32, tag="wcol")
                nc.vector.tensor_tensor(out=wcol[:], in0=sg[:], in1=sel[:],
                                        op=ALU.mult)
                nc.vector.tensor_scalar(out=wcol[:], in0=wcol[:],
                                        scalar1=rec[:, 0:1], scalar2=None,
                                        op0=ALU.mult)
                # spread expert cols 32 apart so the transposed rows land on
                # legal partition bases (0, 32), then transpose
                wc64 = wk.tile([128, 32 * EPC], F32, tag="wc64")
                for e in range(EPC):
                    nc.vector.tensor_copy(out=wc64[:, 32 * e:32 * e + 1],
                                          in_=wcol[:, e:e + 1])
                pwt = pscp.tile([32 * EPC, 128], F32, tag="pg", name="pwt")
                nc.tensor.transpose(out=pwt[:], in_=wc64[:], identity=ident[:])
                for e in range(EPC):
                    nc.vector.tensor_copy(out=wrow[e][:, isl],
                                          in_=pwt[32 * e:32 * e + 1, :])

            # broadcast combine weights across partitions: [1,T] -> [128,T]
            wbc = []
            for e in range(EPC):
                t_ = wp.tile([128, T], BF16, tag=f"wbc{e}")
                nc.gpsimd.partition_broadcast(t_[:], wrow[e][:, :])
                wbc.append(t_)

            # ---- routed experts: h = silu(x@w1t) * (x@w3t) * gate ----------
            hsb = [[wp.tile([128, T], BF16, tag=f"hsb{e}_{m}", name=f"hsb{e}_{m}") for m in range(MI)]
                   for e in range(EPC)]
            hss = wp.tile([128, T], BF16, tag="hss")

            for e in range(EPC):
                for m in range(MI):
                    msl = slice(m * 128, (m + 1) * 128)
                    for n in range(NT):
                        nsl = slice(n * 512, (n + 1) * 512)
                        ps1 = pp.tile([128, 512], F32, tag="ps1")
                        ps3 = pp.tile([128, 512], F32, tag="ps3")
                        for k in range(KD):
                            nc.tensor.matmul(ps1[:], lhsT=w1s[e][k][:, msl],
                                             rhs=xtb[k][:, nsl],
                                             start=(k == 0), stop=(k == KD - 1))
                        for k in range(KD):
                            nc.tensor.matmul(ps3[:], lhsT=w3s[e][k][:, msl],
                                             rhs=xtb[k][:, nsl],
                                             start=(k == 0), stop=(k == KD - 1))
                        t1 = wk.tile([128, 512], BF16, tag="t1")
                        nc.scalar.activation(out=t1[:], in_=ps1[:], func=AF.Silu)
                        tp = wk.tile([128, 512], BF16, tag="tp")
                        nc.vector.tensor_tensor(out=tp[:], in0=t1[:], in1=ps3[:],
                                                op=ALU.mult)
                        nc.vector.tensor_tensor(out=hsb[e][m][:, nsl], in0=tp[:],
                                                in1=wbc[e][:, nsl], op=ALU.mult)

            # ---- shared expert slice ---------------------------------------
            for n in range(NT):
                nsl = slice(n * 512, (n + 1) * 512)
                ps1 = pp.tile([128, 512], F32, tag="ps1")
                ps3 = pp.tile([128, 512], F32, tag="ps3")
                for k in range(KD):
                    nc.tensor.matmul(ps1[:], lhsT=sw1s[k][:], rhs=xtb[k][:, nsl],
                                     start=(k == 0), stop=(k == KD - 1))
                for k in range(KD):
                    nc.tensor.matmul(ps3[:], lhsT=sw3s[k][:], rhs=xtb[k][:, nsl],
                                     start=(k == 0), stop=(k == KD - 1))
                t1 = wk.tile([128, 512], BF16, tag="t1")
                nc.scalar.activation(out=t1[:], in_=ps1[:], func=AF.Silu)
                nc.vector.tensor_tensor(out=hss[:, nsl], in0=t1[:], in1=ps3[:],
                                        op=ALU.mult)

            # ---- y[d, t] = shared + sum_e h_e @ w2t_e ----------------------
            # 4 d-chunked ReduceScatters so the collective overlaps with the
            # remaining y-phase compute; host reassembles the d-interleaving.
            NCH = 4
            DPC = KD // NCH  # d-chunks (of 128) per RS chunk
            y_dram = dp.tile([DIM, T], BF16)
            y_rs = [dp.tile([DIM // N_CORES // NCH, T], BF16, name=f"y_rs{j}")
                    for j in range(NCH)]
            for j in range(NCH):
                for d in range(j * DPC, (j + 1) * DPC):
                    dsl = slice(d * 128, (d + 1) * 128)
                    ysb = wk.tile([128, T], BF16, tag="ysb")
                    for n in range(NT):
                        nsl = slice(n * 512, (n + 1) * 512)
                        psy = pp.tile([128, 512], F32, tag="psy")
                        nc.tensor.matmul(psy[:], lhsT=sw2s[:, dsl],
                                         rhs=hss[:, nsl], start=True, stop=False)
                        for e in range(EPC):
                            for m in range(MI):
                                nc.tensor.matmul(
                                    psy[:], lhsT=w2s[e][m][:, dsl],
                                    rhs=hsb[e][m][:, nsl],
                                    start=False,
                                    stop=(e == EPC - 1 and m == MI - 1))
                        nc.vector.tensor_copy(out=ysb[:, nsl], in_=psy[:])
                    nc.sync.dma_start(out=y_dram[dsl, :], in_=ysb[:])
                rs_rows = DIM // NCH
                nc.gpsimd.collective_compute(
                    "ReduceScatter",
                    ALU.add,
                    replica_groups=[list(range(N_CORES))],
                    ins=[y_dram[j * rs_rows:(j + 1) * rs_rows, :]],
                    outs=[y_rs[j].opt()],
                )
                orow = DIM // N_CORES // NCH
                nc.gpsimd.dma_start(out=out_d[j * orow:(j + 1) * orow, :],
                                    in_=y_rs[j][:])

    nc.compile()
    return nc


def kernel(x, gate_w, w1, w2, w3, sw1, sw2, sw3):
    global last_exec_time_ns

    xt = np.ascontiguousarray(
        np.asarray(x, np.float32).reshape(T, DIM).T)      # [D, T] fp32
    xtb = xt.astype(BF)
    gate_w = np.asarray(gate_w, np.float32)
    w1 = np.asarray(w1, np.float32)
    w2 = np.asarray(w2, np.float32)
    w3 = np.asarray(w3, np.float32)
    sw1 = np.asarray(sw1, np.float32)
    sw2 = np.asarray(sw2, np.float32)
    sw3 = np.asarray(sw3, np.float32)

    in_maps = []
    for c in range(N_CORES):
        mine = [EPC * c + j for j in range(EPC)]
        perm = mine + [e for e in range(E) if e not in mine]
        gwt = np.ascontiguousarray(gate_w[perm].T)        # [D, E] fp32
        w1t = np.stack([np.ascontiguousarray(w1[e].T) for e in mine]).astype(BF)
        w3t = np.stack([np.ascontiguousarray(w3[e].T) for e in mine]).astype(BF)
        w2t = np.stack([np.ascontiguousarray(w2[e].T) for e in mine]).astype(BF)
        ssl = slice(SIC * c, SIC * (c + 1))
        sw1t = np.ascontiguousarray(sw1[ssl, :].T).astype(BF)  # [D, SIC]
        sw3t = np.ascontiguousarray(sw3[ssl, :].T).astype(BF)
        sw2t = np.ascontiguousarray(sw2[:, ssl].T).astype(BF)  # [SIC, D]
        in_maps.append({
            "xtf": xt, "xtb": xtb, "gwt": gwt,
            "w1t": w1t, "w3t": w3t, "w2t": w2t,
            "sw1t": sw1t, "sw3t": sw3t, "sw2t": sw2t,
        })

    if "nc" not in _cached:
        _cached["nc"] = _build()
    nc = _cached["nc"]

    res = run_bass_kernel_spmd(nc, in_maps, core_ids=list(range(N_CORES)))
    last_exec_time_ns = res.exec_time_ns

    # reassemble the d-interleaved RS chunk layout:
    # core c, out rows [j*32:(j+1)*32] = global d rows [j*256+c*32, ...+32)
    NCH, orow = 4, DIM // N_CORES // 4
    yt = np.empty((DIM, T), np.float32)
    for c in range(N_CORES):
        o = res.results[c]["out"].astype(np.float32)
        for j in range(NCH):
            g = j * (DIM // NCH) + c * orow
            yt[g:g + orow] = o[j * orow:(j + 1) * orow]
    return np.ascontiguousarray(yt.T).reshape(B, S, DIM).astype(np.float32)


# revision 9
# speedup vs baseline: 1.1886x; 1.1263x over previous
"""Trainium2 Bass kernel for nn_MoE_66803921322559.

Top-2-of-16 MoE (T=2048 tokens, D=1024, INTER=512) + shared expert
(SHARED_INTER=1024), expert-parallel over 8 NeuronCores:

  - core c owns experts (2c, 2c+1); gate computed on-device (double-bf16
    logits = xh@gh + xl@gh + xh@gl, exact enough that top-2 selection
    matches fp32; sigmoid/normalize for combine weights)
  - routed experts + shared-expert slice computed in bf16 on device;
    shared expert is scheduled first so the gate pipeline latency hides
  - partial outputs y [D, T] summed across cores with 4 d-chunked
    ReduceScatters overlapped with the y-phase matmuls; host reassembles.
"""

import os
import sys
import types

import numpy as np

sys.path.insert(0, "/opt/trn_rl_repo")

import ml_dtypes

BF = ml_dtypes.bfloat16

B, S, DIM = 2, 1024, 1024
E, K, INTER = 16, 2, 512
T = B * S
N_CORES = 8
EPC = E // N_CORES          # experts per core
SIC = 2 * INTER // N_CORES  # shared-inter slice per core (128)

KD = DIM // 128             # 8 contraction chunks over D
NT = T // 512               # 4 token chunks of 512
MI = INTER // 128           # 4 inter chunks per expert
NTILE = T // 128            # 16 token tiles of 128
NCH = 4                     # ReduceScatter d-chunks


def _install_ntff_hook():
    """Provide antenv.axon_hooks (missing in this container) so
    run_bass_kernel_spmd(trace=True) can capture NTFF profiles via axon."""
    try:
        import antenv
        if hasattr(antenv, "axon_hooks"):
            return
        from trn_agent_boot.trn_boot import _ntff_profile_via_ctypes
        hook = _ntff_profile_via_ctypes("/opt/axon/libaxon_pjrt.so")
        mod = types.ModuleType("antenv.axon_hooks")
        mod._hook = hook
        mod.get_axon_ntff_profile_hook = lambda: mod._hook
        mod.set_axon_ntff_profile_hook = lambda h: setattr(mod, "_hook", h)
        sys.modules["antenv.axon_hooks"] = mod
        antenv.axon_hooks = mod
    except Exception:
        pass


_install_ntff_hook()

from concourse import bacc, bass, mybir, tile  # noqa: E402
from concourse.bass_utils import run_bass_kernel_spmd  # noqa: E402
from concourse.masks import make_identity  # noqa: E402

F32 = mybir.dt.float32
BF16 = mybir.dt.bfloat16
AF = mybir.ActivationFunctionType
ALU = mybir.AluOpType

last_exec_time_ns = None
_cached = {}


def _build():
    nc = bacc.Bacc("TRN2", target_bir_lowering=False, debug=False,
                   num_devices=N_CORES)

    xtb_d = nc.dram_tensor("xtb", [DIM, T], BF16, kind="ExternalInput").ap()
    xtl_d = nc.dram_tensor("xtl", [DIM, T], BF16, kind="ExternalInput").ap()
    gh_d = nc.dram_tensor("gh", [DIM, E], BF16, kind="ExternalInput").ap()
    gl_d = nc.dram_tensor("gl", [DIM, E], BF16, kind="ExternalInput").ap()
    w1t_d = nc.dram_tensor("w1t", [EPC, DIM, INTER], BF16, kind="ExternalInput").ap()
    w3t_d = nc.dram_tensor("w3t", [EPC, DIM, INTER], BF16, kind="ExternalInput").ap()
    w2t_d = nc.dram_tensor("w2t", [EPC, INTER, DIM], BF16, kind="ExternalInput").ap()
    sw1t_d = nc.dram_tensor("sw1t", [DIM, SIC], BF16, kind="ExternalInput").ap()
    sw3t_d = nc.dram_tensor("sw3t", [DIM, SIC], BF16, kind="ExternalInput").ap()
    sw2t_d = nc.dram_tensor("sw2t", [SIC, DIM], BF16, kind="ExternalInput").ap()
    out_d = nc.dram_tensor("out", [DIM // N_CORES, T], BF16,
                           kind="ExternalOutput").ap()

    with tile.TileContext(nc) as tc:
        with (
            tc.tile_pool(name="wpool", bufs=1) as wp,
            tc.tile_pool(name="work", bufs=3) as wk,
            tc.tile_pool(name="psum", bufs=2, space="PSUM") as pp,
            tc.tile_pool(name="psc", bufs=2, space="PSUM") as pscp,
            tc.tile_pool(name="dram", bufs=1, space="DRAM") as dp,
        ):
            # ---- persistent SBUF loads, in consumption order ---------------
            # sync ring: gate inputs first; scalar ring: expert weights
            xtb, xtl, ghs, gls = [], [], [], []
            for k in range(KD):
                ksl = slice(k * 128, (k + 1) * 128)
                t_ = wp.tile([128, T], BF16, tag=f"xtb{k}", name=f"xtb{k}")
                nc.sync.dma_start(out=t_[:], in_=xtb_d[ksl, :])
                xtb.append(t_)
                t_ = wp.tile([128, E], BF16, tag=f"gh{k}", name=f"gh{k}")
                nc.sync.dma_start(out=t_[:], in_=gh_d[ksl, :])
                ghs.append(t_)
                t_ = wp.tile([128, E], BF16, tag=f"gl{k}", name=f"gl{k}")
                nc.sync.dma_start(out=t_[:], in_=gl_d[ksl, :])
                gls.append(t_)
            for k in range(KD):
                ksl = slice(k * 128, (k + 1) * 128)
                t_ = wp.tile([128, T], BF16, tag=f"xtl{k}", name=f"xtl{k}")
                nc.sync.dma_start(out=t_[:], in_=xtl_d[ksl, :])
                xtl.append(t_)
            sw1s, sw3s = [], []
            for k in range(KD):
                ksl = slice(k * 128, (k + 1) * 128)
                t_ = wp.tile([128, SIC], BF16, tag=f"sw1s{k}", name=f"sw1s{k}")
                nc.scalar.dma_start(out=t_[:], in_=sw1t_d[ksl, :])
                sw1s.append(t_)
                t_ = wp.tile([128, SIC], BF16, tag=f"sw3s{k}", name=f"sw3s{k}")
                nc.scalar.dma_start(out=t_[:], in_=sw3t_d[ksl, :])
                sw3s.append(t_)
            w1s = [[None] * KD for _ in range(EPC)]
            w3s = [[None] * KD for _ in range(EPC)]
            for e in range(EPC):
                for k in range(KD):
                    ksl = slice(k * 128, (k + 1) * 128)
                    t_ = wp.tile([128, INTER], BF16, tag=f"w1s{e}_{k}",
                                 name=f"w1s{e}_{k}")
                    nc.scalar.dma_start(out=t_[:], in_=w1t_d[e, ksl, :])
                    w1s[e][k] = t_
                    t_ = wp.tile([128, INTER], BF16, tag=f"w3s{e}_{k}",
                                 name=f"w3s{e}_{k}")
                    nc.scalar.dma_start(out=t_[:], in_=w3t_d[e, ksl, :])
                    w3s[e][k] = t_
            w2s = [[None] * MI for _ in range(EPC)]
            for e in range(EPC):
                for m in range(MI):
                    msl = slice(m * 128, (m + 1) * 128)
                    t_ = wp.tile([128, DIM], BF16, tag=f"w2s{e}_{m}",
                                 name=f"w2s{e}_{m}")
                    nc.scalar.dma_start(out=t_[:], in_=w2t_d[e, msl, :])
                    w2s[e][m] = t_
            sw2s = wp.tile([128, DIM], BF16, tag="sw2s")
            nc.scalar.dma_start(out=sw2s[:], in_=sw2t_d[:, :])

            ident = wp.tile([128, 128], F32, tag="ident")
            make_identity(nc, ident[:])

            # ---- gate: double-bf16 logits [E, T] ---------------------------
            logits = wp.tile([E, T], F32, tag="logits")
            for n in range(NT):
                nsl = slice(n * 512, (n + 1) * 512)
                psc = pscp.tile([E, 512], F32, tag="pg", name="psc")
                for k in range(KD):
                    nc.tensor.matmul(psc[:], lhsT=ghs[k][:], rhs=xtb[k][:, nsl],
                                     start=(k == 0), stop=False)
                for k in range(KD):
                    nc.tensor.matmul(psc[:], lhsT=ghs[k][:], rhs=xtl[k][:, nsl],
                                     start=False, stop=False)
                for k in range(KD):
                    nc.tensor.matmul(psc[:], lhsT=gls[k][:], rhs=xtb[k][:, nsl],
                                     start=False, stop=(k == KD - 1))
                nc.vector.tensor_copy(out=logits[:, nsl], in_=psc[:])

            # batched gate: transpose logits, top-2, combine weights
            lg_all = wp.tile([128, NTILE * E], F32, tag="lg_all")
            mx_all = wp.tile([128, NTILE * 8], F32, tag="mx_all")
            for i in range(NTILE):
                isl = slice(i * 128, (i + 1) * 128)
                ptr = pscp.tile([128, E], F32, tag="pg", name="ptr")
                nc.tensor.transpose(out=ptr[:], in_=logits[:, isl],
                                    identity=ident[:E, :E])
                nc.vector.tensor_copy(out=lg_all[:, i * E:(i + 1) * E],
                                      in_=ptr[:])
            for i in range(NTILE):
                nc.vector.max(out=mx_all[:, i * 8:(i + 1) * 8],
                              in_=lg_all[:, i * E:(i + 1) * E])
            # denominator = sigmoid(l1) + sigmoid(l2); rec = 1/den
            mx3 = mx_all[:].rearrange("p (i c) -> p i c", c=8)
            lg3 = lg_all[:].rearrange("p (i c) -> p i c", c=E)
            s12 = wk.tile([128, 2 * NTILE], F32, tag="s12")
            nc.scalar.activation(out=s12[:], in_=mx3[:, :, 0:2], func=AF.Sigmoid)
            s12v = s12[:].rearrange("p (i c) -> p i c", c=2)
            den = wk.tile([128, NTILE], F32, tag="den")
            nc.vector.tensor_tensor(out=den[:], in0=s12v[:, :, 0],
                                    in1=s12v[:, :, 1], op=ALU.add)
            rec = wk.tile([128, NTILE], F32, tag="rec")
            nc.vector.reciprocal(out=rec[:], in_=den[:])

            wexp = []
            for e in range(EPC):
                sel = wk.tile([128, NTILE], F32, tag=f"sel{e}", name=f"sel{e}")
                nc.vector.tensor_tensor(out=sel[:], in0=lg3[:, :, e],
                                        in1=mx3[:, :, 1], op=ALU.is_ge)
                sg = wk.tile([128, NTILE], F32, tag=f"sg{e}", name=f"sg{e}")
                nc.scalar.activation(out=sg[:], in_=lg3[:, :, e], func=AF.Sigmoid)
                we = wk.tile([128, NTILE], F32, tag=f"wexp{e}", name=f"wexp{e}")
                nc.vector.tensor_tensor(out=we[:], in0=sg[:], in1=sel[:],
                                        op=ALU.mult)
                nc.vector.tensor_tensor(out=we[:], in0=we[:], in1=rec[:],
                                        op=ALU.mult)
                wexp.append(we)

            # transpose per-token weights into rows, then partition-broadcast
            wbc = []
            for e in range(EPC):
                wrow = wp.tile([1, T], BF16, tag=f"wrow{e}", name=f"wrow{e}")
                for i in range(NTILE):
                    pwt = pscp.tile([1, 128], F32, tag="pg", name="pwt")
                    nc.tensor.transpose(out=pwt[:], in_=wexp[e][:, i:i + 1],
                                        identity=ident[:])
                    nc.vector.tensor_copy(out=wrow[:, i * 128:(i + 1) * 128],
                                          in_=pwt[:])
                t_ = wp.tile([128, T], BF16, tag=f"wbc{e}", name=f"wbc{e}")
                nc.gpsimd.partition_broadcast(t_[:], wrow[:, :])
                wbc.append(t_)

            # ---- shared expert slice first (no gate dependency) ------------
            hss = wp.tile([128, T], BF16, tag="hss")
            for n in range(NT):
                nsl = slice(n * 512, (n + 1) * 512)
                ps1 = pp.tile([128, 512], F32, tag="ps1", name="ps1")
                ps3 = pp.tile([128, 512], F32, tag="ps3", name="ps3")
                for k in range(KD):
                    nc.tensor.matmul(ps1[:], lhsT=sw1s[k][:], rhs=xtb[k][:, nsl],
                                     start=(k == 0), stop=(k == KD - 1))
                for k in range(KD):
                    nc.tensor.matmul(ps3[:], lhsT=sw3s[k][:], rhs=xtb[k][:, nsl],
                                     start=(k == 0), stop=(k == KD - 1))
                t1 = wk.tile([128, 512], BF16, tag="t1")
                nc.scalar.activation(out=t1[:], in_=ps1[:], func=AF.Silu)
                nc.vector.tensor_tensor(out=hss[:, nsl], in0=t1[:], in1=ps3[:],
                                        op=ALU.mult)

            # ---- routed experts: h = silu(x@w1t) * (x@w3t) * gate ----------
            hsb = [[wp.tile([128, T], BF16, tag=f"hsb{e}_{m}", name=f"hsb{e}_{m}")
                    for m in range(MI)] for e in range(EPC)]
            for e in range(EPC):
                for m in range(MI):
                    msl = slice(m * 128, (m + 1) * 128)
                    for n in range(NT):
                        nsl = slice(n * 512, (n + 1) * 512)
                        ps1 = pp.tile([128, 512], F32, tag="ps1", name="ps1")
                        ps3 = pp.tile([128, 512], F32, tag="ps3", name="ps3")
                        for k in range(KD):
                            nc.tensor.matmul(ps1[:], lhsT=w1s[e][k][:, msl],
                                             rhs=xtb[k][:, nsl],
                                             start=(k == 0), stop=(k == KD - 1))
                        for k in range(KD):
                            nc.tensor.matmul(ps3[:], lhsT=w3s[e][k][:, msl],
                                             rhs=xtb[k][:, nsl],
                                             start=(k == 0), stop=(k == KD - 1))
                        t1 = wk.tile([128, 512], BF16, tag="t1")
                        nc.scalar.activation(out=t1[:], in_=ps1[:], func=AF.Silu)
                        tp = wk.tile([128, 512], BF16, tag="tp")
                        nc.vector.tensor_tensor(out=tp[:], in0=t1[:], in1=ps3[:],
                                                op=ALU.mult)
                        nc.vector.tensor_tensor(out=hsb[e][m][:, nsl], in0=tp[:],
                                                in1=wbc[e][:, nsl], op=ALU.mult)

            # ---- y[d, t] = shared + sum_e h_e @ w2t_e, chunked RS ----------
            DPC = KD // NCH
            y_dram = dp.tile([DIM, T], BF16)
            y_rs = [dp.tile([DIM // N_CORES // NCH, T], BF16, name=f"y_rs{j}")
                    for j in range(NCH)]
            for j in range(NCH):
                for d in range(j * DPC, (j + 1) * DPC):
                    dsl = slice(d * 128, (d + 1) * 128)
                    ysb = wk.tile([128, T], BF16, tag="ysb")
                    for n in range(NT):
                        nsl = slice(n * 512, (n + 1) * 512)
                        psy = pp.tile([128, 512], F32, tag="psy", name="psy")
                        nc.tensor.matmul(psy[:], lhsT=sw2s[:, dsl],
                                         rhs=hss[:, nsl], start=True, stop=False)
                        for e in range(EPC):
                            for m in range(MI):
                                nc.tensor.matmul(
                                    psy[:], lhsT=w2s[e][m][:, dsl],
                                    rhs=hsb[e][m][:, nsl],
                                    start=False,
                                    stop=(e == EPC - 1 and m == MI - 1))
                        nc.vector.tensor_copy(out=ysb[:, nsl], in_=psy[:])
                    nc.sync.dma_start(out=y_dram[dsl, :], in_=ysb[:])
                rs_rows = DIM // NCH
                nc.gpsimd.collective_compute(
                    "ReduceScatter",
                    ALU.add,
                    replica_groups=[list(range(N_CORES))],
                    ins=[y_dram[j * rs_rows:(j + 1) * rs_rows, :]],
                    outs=[y_rs[j].opt()],
                )
                orow = DIM // N_CORES // NCH
                nc.gpsimd.dma_start(out=out_d[j * orow:(j + 1) * orow, :],
                                    in_=y_rs[j][:])

    nc.compile()
    return nc


def kernel(x, gate_w, w1, w2, w3, sw1, sw2, sw3):
    global last_exec_time_ns

    xt = np.ascontiguousarray(
        np.asarray(x, np.float32).reshape(T, DIM).T)      # [D, T] fp32
    xtb = xt.astype(BF)
    xtl = (xt - xtb.astype(np.float32)).astype(BF)
    gate_w = np.asarray(gate_w, np.float32)
    w1 = np.asarray(w1, np.float32)
    w2 = np.asarray(w2, np.float32)
    w3 = np.asarray(w3, np.float32)
    sw1 = np.asarray(sw1, np.float32)
    sw2 = np.asarray(sw2, np.float32)
    sw3 = np.asarray(sw3, np.float32)

    in_maps = []
    for c in range(N_CORES):
        mine = [EPC * c + j for j in range(EPC)]
        perm = mine + [e for e in range(E) if e not in mine]
        gwt = np.ascontiguousarray(gate_w[perm].T)        # [D, E] fp32
        gh = gwt.astype(BF)
        gl = (gwt - gh.astype(np.float32)).astype(BF)
        w1t = np.stack([np.ascontiguousarray(w1[e].T) for e in mine]).astype(BF)
        w3t = np.stack([np.ascontiguousarray(w3[e].T) for e in mine]).astype(BF)
        w2t = np.stack([np.ascontiguousarray(w2[e].T) for e in mine]).astype(BF)
        ssl = slice(SIC * c, SIC * (c + 1))
        sw1t = np.ascontiguousarray(sw1[ssl, :].T).astype(BF)  # [D, SIC]
        sw3t = np.ascontiguousarray(sw3[ssl, :].T).astype(BF)
        sw2t = np.ascontiguousarray(sw2[:, ssl].T).astype(BF)  # [SIC, D]
        in_maps.append({
            "xtb": xtb, "xtl": xtl, "gh": gh, "gl": gl,
            "w1t": w1t, "w3t": w3t, "w2t": w2t,
            "sw1t": sw1t, "sw3t": sw3t, "sw2t": sw2t,
        })

    if "nc" not in _cached:
        _cached["nc"] = _build()
    nc = _cached["nc"]

    res = run_bass_kernel_spmd(nc, in_maps, core_ids=list(range(N_CORES)))
    last_exec_time_ns = res.exec_time_ns

    # reassemble the d-interleaved RS chunk layout:
    # core c, out rows [j*32:(j+1)*32] = global d rows [j*256+c*32, ...+32)
    orow = DIM // N_CORES // NCH
    yt = np.empty((DIM, T), np.float32)
    for c in range(N_CORES):
        o = res.results[c]["out"].astype(np.float32)
        for j in range(NCH):
            g = j * (DIM // NCH) + c * orow
            yt[g:g + orow] = o[j * orow:(j + 1) * orow]
    return np.ascontiguousarray(yt.T).reshape(B, S, DIM).astype(np.float32)


# revision 10
# speedup vs baseline: 1.2571x; 1.0576x over previous
"""Trainium2 Bass kernel for nn_MoE_66803921322559.

Top-2-of-16 MoE (T=2048 tokens, D=1024, INTER=512) + shared expert
(SHARED_INTER=1024), expert-parallel over 8 NeuronCores:

  - core c owns experts (2c, 2c+1); gate computed on-device (double-bf16
    logits = xh@gh + xl@gh + xh@gl, exact enough that top-2 selection
    matches fp32; sigmoid/normalize for combine weights)
  - routed experts + shared-expert slice computed in bf16 on device;
    shared expert is scheduled first so the gate pipeline latency hides
  - partial outputs y [D, T] summed across cores with 4 d-chunked
    ReduceScatters overlapped with the y-phase matmuls; host reassembles.
"""

import os
import sys
import types

import numpy as np

sys.path.insert(0, "/opt/trn_rl_repo")

import ml_dtypes

BF = ml_dtypes.bfloat16

B, S, DIM = 2, 1024, 1024
E, K, INTER = 16, 2, 512
T = B * S
N_CORES = 8
EPC = E // N_CORES          # experts per core
SIC = 2 * INTER // N_CORES  # shared-inter slice per core (128)

KD = DIM // 128             # 8 contraction chunks over D
NT = T // 512               # 4 token chunks of 512
MI = INTER // 128           # 4 inter chunks per expert
NTILE = T // 128            # 16 token tiles of 128
NCH = 4                     # ReduceScatter d-chunks


def _install_ntff_hook():
    """Provide antenv.axon_hooks (missing in this container) so
    run_bass_kernel_spmd(trace=True) can capture NTFF profiles via axon."""
    try:
        import antenv
        if hasattr(antenv, "axon_hooks"):
            return
        from trn_agent_boot.trn_boot import _ntff_profile_via_ctypes
        hook = _ntff_profile_via_ctypes("/opt/axon/libaxon_pjrt.so")
        mod = types.ModuleType("antenv.axon_hooks")
        mod._hook = hook
        mod.get_axon_ntff_profile_hook = lambda: mod._hook
        mod.set_axon_ntff_profile_hook = lambda h: setattr(mod, "_hook", h)
        sys.modules["antenv.axon_hooks"] = mod
        antenv.axon_hooks = mod
    except Exception:
        pass


_install_ntff_hook()

from concourse import bacc, bass, mybir, tile  # noqa: E402
from concourse.bass_utils import run_bass_kernel_spmd  # noqa: E402
from concourse.masks import make_identity  # noqa: E402

F32 = mybir.dt.float32
BF16 = mybir.dt.bfloat16
AF = mybir.ActivationFunctionType
ALU = mybir.AluOpType

last_exec_time_ns = None
_cached = {}


def _build():
    nc = bacc.Bacc("TRN2", target_bir_lowering=False, debug=False,
                   num_devices=N_CORES)

    xtb_d = nc.dram_tensor("xtb", [DIM, T], BF16, kind="ExternalInput").ap()
    xtl_d = nc.dram_tensor("xtl", [DIM, T], BF16, kind="ExternalInput").ap()
    gh_d = nc.dram_tensor("gh", [DIM, E], BF16, kind="ExternalInput").ap()
    gl_d = nc.dram_tensor("gl", [DIM, E], BF16, kind="ExternalInput").ap()
    w1t_d = nc.dram_tensor("w1t", [EPC, DIM, INTER], BF16, kind="ExternalInput").ap()
    w3t_d = nc.dram_tensor("w3t", [EPC, DIM, INTER], BF16, kind="ExternalInput").ap()
    w2t_d = nc.dram_tensor("w2t", [EPC, INTER, DIM], BF16, kind="ExternalInput").ap()
    sw1t_d = nc.dram_tensor("sw1t", [DIM, SIC], BF16, kind="ExternalInput").ap()
    sw3t_d = nc.dram_tensor("sw3t", [DIM, SIC], BF16, kind="ExternalInput").ap()
    sw2t_d = nc.dram_tensor("sw2t", [SIC, DIM], BF16, kind="ExternalInput").ap()
    out_d = nc.dram_tensor("out", [DIM // N_CORES, T], BF16,
                           kind="ExternalOutput").ap()

    with tile.TileContext(nc) as tc:
        with (
            tc.tile_pool(name="wpool", bufs=1) as wp,
            tc.tile_pool(name="work", bufs=3) as wk,
            tc.tile_pool(name="psum", bufs=2, space="PSUM") as pp,
            tc.tile_pool(name="psc", bufs=2, space="PSUM") as pscp,
            tc.tile_pool(name="dram", bufs=1, space="DRAM") as dp,
        ):
            # ---- persistent SBUF loads, in consumption order ---------------
            # sync ring: gate inputs first; scalar ring: expert weights
            xtb, xtl, ghs, gls = [], [], [], []
            for k in range(KD):
                ksl = slice(k * 128, (k + 1) * 128)
                t_ = wp.tile([128, T], BF16, tag=f"xtb{k}", name=f"xtb{k}")
                nc.sync.dma_start(out=t_[:], in_=xtb_d[ksl, :])
                xtb.append(t_)
                t_ = wp.tile([128, E], BF16, tag=f"gh{k}", name=f"gh{k}")
                nc.sync.dma_start(out=t_[:], in_=gh_d[ksl, :])
                ghs.append(t_)
                t_ = wp.tile([128, E], BF16, tag=f"gl{k}", name=f"gl{k}")
                nc.sync.dma_start(out=t_[:], in_=gl_d[ksl, :])
                gls.append(t_)
            for k in range(KD):
                ksl = slice(k * 128, (k + 1) * 128)
                t_ = wp.tile([128, T], BF16, tag=f"xtl{k}", name=f"xtl{k}")
                nc.sync.dma_start(out=t_[:], in_=xtl_d[ksl, :])
                xtl.append(t_)
            sw1s, sw3s = [], []
            for k in range(KD):
                ksl = slice(k * 128, (k + 1) * 128)
                t_ = wp.tile([128, SIC], BF16, tag=f"sw1s{k}", name=f"sw1s{k}")
                nc.scalar.dma_start(out=t_[:], in_=sw1t_d[ksl, :])
                sw1s.append(t_)
                t_ = wp.tile([128, SIC], BF16, tag=f"sw3s{k}", name=f"sw3s{k}")
                nc.scalar.dma_start(out=t_[:], in_=sw3t_d[ksl, :])
                sw3s.append(t_)
            w1s = [[None] * KD for _ in range(EPC)]
            w3s = [[None] * KD for _ in range(EPC)]
            for e in range(EPC):
                for k in range(KD):
                    ksl = slice(k * 128, (k + 1) * 128)
                    t_ = wp.tile([128, INTER], BF16, tag=f"w1s{e}_{k}",
                                 name=f"w1s{e}_{k}")
                    nc.scalar.dma_start(out=t_[:], in_=w1t_d[e, ksl, :])
                    w1s[e][k] = t_
                    t_ = wp.tile([128, INTER], BF16, tag=f"w3s{e}_{k}",
                                 name=f"w3s{e}_{k}")
                    nc.scalar.dma_start(out=t_[:], in_=w3t_d[e, ksl, :])
                    w3s[e][k] = t_
            w2s = [[None] * MI for _ in range(EPC)]
            for e in range(EPC):
                for m in range(MI):
                    msl = slice(m * 128, (m + 1) * 128)
                    t_ = wp.tile([128, DIM], BF16, tag=f"w2s{e}_{m}",
                                 name=f"w2s{e}_{m}")
                    nc.scalar.dma_start(out=t_[:], in_=w2t_d[e, msl, :])
                    w2s[e][m] = t_
            sw2s = wp.tile([128, DIM], BF16, tag="sw2s")
            nc.scalar.dma_start(out=sw2s[:], in_=sw2t_d[:, :])

            ident = wp.tile([128, 128], F32, tag="ident")
            make_identity(nc, ident[:])

            # ---- gate: double-bf16 logits [E, T] ---------------------------
            logits = wp.tile([E, T], F32, tag="logits")
            for n in range(NT):
                nsl = slice(n * 512, (n + 1) * 512)
                psc = pscp.tile([E, 512], F32, tag="pg", name="psc")
                for k in range(KD):
                    nc.tensor.matmul(psc[:], lhsT=ghs[k][:], rhs=xtb[k][:, nsl],
                                     start=(k == 0), stop=False)
                for k in range(KD):
                    nc.tensor.matmul(psc[:], lhsT=ghs[k][:], rhs=xtl[k][:, nsl],
                                     start=False, stop=False)
                for k in range(KD):
                    nc.tensor.matmul(psc[:], lhsT=gls[k][:], rhs=xtb[k][:, nsl],
                                     start=False, stop=(k == KD - 1))
                nc.vector.tensor_copy(out=logits[:, nsl], in_=psc[:])

            # batched gate: transpose logits, top-2, combine weights
            lg_all = wp.tile([128, NTILE * E], F32, tag="lg_all")
            mx_all = wp.tile([128, NTILE * 8], F32, tag="mx_all")
            for i in range(NTILE):
                isl = slice(i * 128, (i + 1) * 128)
                ptr = pscp.tile([128, E], F32, tag="pg", name="ptr")
                nc.tensor.transpose(out=ptr[:], in_=logits[:, isl],
                                    identity=ident[:E, :E])
                nc.vector.tensor_copy(out=lg_all[:, i * E:(i + 1) * E],
                                      in_=ptr[:])
            for i in range(NTILE):
                nc.vector.max(out=mx_all[:, i * 8:(i + 1) * 8],
                              in_=lg_all[:, i * E:(i + 1) * E])
            # denominator = sigmoid(l1) + sigmoid(l2); rec = 1/den
            mx3 = mx_all[:].rearrange("p (i c) -> p i c", c=8)
            lg3 = lg_all[:].rearrange("p (i c) -> p i c", c=E)
            s12 = wk.tile([128, 2 * NTILE], F32, tag="s12")
            nc.scalar.activation(out=s12[:], in_=mx3[:, :, 0:2], func=AF.Sigmoid)
            s12v = s12[:].rearrange("p (i c) -> p i c", c=2)
            den = wk.tile([128, NTILE], F32, tag="den")
            nc.vector.tensor_tensor(out=den[:], in0=s12v[:, :, 0],
                                    in1=s12v[:, :, 1], op=ALU.add)
            rec = wk.tile([128, NTILE], F32, tag="rec")
            nc.vector.reciprocal(out=rec[:], in_=den[:])

            wexp = []
            for e in range(EPC):
                sel = wk.tile([128, NTILE], F32, tag=f"sel{e}", name=f"sel{e}")
                nc.vector.tensor_tensor(out=sel[:], in0=lg3[:, :, e],
                                        in1=mx3[:, :, 1], op=ALU.is_ge)
                sg = wk.tile([128, NTILE], F32, tag=f"sg{e}", name=f"sg{e}")
                nc.scalar.activation(out=sg[:], in_=lg3[:, :, e], func=AF.Sigmoid)
                we = wk.tile([128, NTILE], F32, tag=f"wexp{e}", name=f"wexp{e}")
                nc.vector.tensor_tensor(out=we[:], in0=sg[:], in1=sel[:],
                                        op=ALU.mult)
                nc.vector.tensor_tensor(out=we[:], in0=we[:], in1=rec[:],
                                        op=ALU.mult)
                wexp.append(we)

            # transpose per-token weights into rows, then partition-broadcast
            wbc = []
            for e in range(EPC):
                wrow = wp.tile([1, T], BF16, tag=f"wrow{e}", name=f"wrow{e}")
                for i in range(NTILE):
                    pwt = pscp.tile([1, 128], F32, tag="pg", name="pwt")
                    nc.tensor.transpose(out=pwt[:], in_=wexp[e][:, i:i + 1],
                                        identity=ident[:])
                    nc.vector.tensor_copy(out=wrow[:, i * 128:(i + 1) * 128],
                                          in_=pwt[:])
                t_ = wp.tile([128, T], BF16, tag=f"wbc{e}", name=f"wbc{e}")
                nc.gpsimd.partition_broadcast(t_[:], wrow[:, :])
                wbc.append(t_)

            # ---- fused pipeline, token-chunk outer: h(n) -> y(n) -> RS(n) --
            hss = wp.tile([128, T], BF16, tag="hss")
            hsb = [[wp.tile([128, T], BF16, tag=f"hsb{e}_{m}", name=f"hsb{e}_{m}")
                    for m in range(MI)] for e in range(EPC)]
            y_ch = [dp.tile([DIM, 512], BF16, name=f"y_ch{n}") for n in range(NT)]
            y_rs = [dp.tile([DIM // N_CORES, 512], BF16, name=f"y_rs{n}")
                    for n in range(NT)]
            for n in range(NT):
                nsl = slice(n * 512, (n + 1) * 512)
                # shared expert slice (no gate dependency) first
                ps1 = pp.tile([128, 512], F32, tag="ps1", name="ps1")
                ps3 = pp.tile([128, 512], F32, tag="ps3", name="ps3")
                for k in range(KD):
                    nc.tensor.matmul(ps1[:], lhsT=sw1s[k][:], rhs=xtb[k][:, nsl],
                                     start=(k == 0), stop=(k == KD - 1))
                for k in range(KD):
                    nc.tensor.matmul(ps3[:], lhsT=sw3s[k][:], rhs=xtb[k][:, nsl],
                                     start=(k == 0), stop=(k == KD - 1))
                t1 = wk.tile([128, 512], BF16, tag="t1")
                nc.scalar.activation(out=t1[:], in_=ps1[:], func=AF.Silu)
                nc.vector.tensor_tensor(out=hss[:, nsl], in0=t1[:], in1=ps3[:],
                                        op=ALU.mult)
                # routed experts
                for e in range(EPC):
                    for m in range(MI):
                        msl = slice(m * 128, (m + 1) * 128)
                        ps1 = pp.tile([128, 512], F32, tag="ps1", name="ps1")
                        ps3 = pp.tile([128, 512], F32, tag="ps3", name="ps3")
                        for k in range(KD):
                            nc.tensor.matmul(ps1[:], lhsT=w1s[e][k][:, msl],
                                             rhs=xtb[k][:, nsl],
                                             start=(k == 0), stop=(k == KD - 1))
                        for k in range(KD):
                            nc.tensor.matmul(ps3[:], lhsT=w3s[e][k][:, msl],
                                             rhs=xtb[k][:, nsl],
                                             start=(k == 0), stop=(k == KD - 1))
                        t1 = wk.tile([128, 512], BF16, tag="t1")
                        nc.scalar.activation(out=t1[:], in_=ps1[:], func=AF.Silu)
                        tp = wk.tile([128, 512], BF16, tag="tp")
                        nc.vector.tensor_tensor(out=tp[:], in0=t1[:], in1=ps3[:],
                                                op=ALU.mult)
                        nc.vector.tensor_tensor(out=hsb[e][m][:, nsl], in0=tp[:],
                                                in1=wbc[e][:, nsl], op=ALU.mult)
                # y for this token chunk, then its ReduceScatter
                for d in range(KD):
                    dsl = slice(d * 128, (d + 1) * 128)
                    psy = pp.tile([128, 512], F32, tag="psy", name="psy")
                    nc.tensor.matmul(psy[:], lhsT=sw2s[:, dsl],
                                     rhs=hss[:, nsl], start=True, stop=False)
                    for e in range(EPC):
                        for m in range(MI):
                            nc.tensor.matmul(
                                psy[:], lhsT=w2s[e][m][:, dsl],
                                rhs=hsb[e][m][:, nsl],
                                start=False,
                                stop=(e == EPC - 1 and m == MI - 1))
                    ysb = wk.tile([128, 512], BF16, tag="ysb")
                    nc.vector.tensor_copy(out=ysb[:], in_=psy[:])
                    nc.sync.dma_start(out=y_ch[n][dsl, :], in_=ysb[:])
                nc.gpsimd.collective_compute(
                    "ReduceScatter",
                    ALU.add,
                    replica_groups=[list(range(N_CORES))],
                    ins=[y_ch[n].opt()],
                    outs=[y_rs[n].opt()],
                )
                nc.gpsimd.dma_start(out=out_d[:, nsl], in_=y_rs[n][:])

    nc.compile()
    return nc


def kernel(x, gate_w, w1, w2, w3, sw1, sw2, sw3):
    global last_exec_time_ns

    xt = np.ascontiguousarray(
        np.asarray(x, np.float32).reshape(T, DIM).T)      # [D, T] fp32
    xtb = xt.astype(BF)
    xtl = (xt - xtb.astype(np.float32)).astype(BF)
    gate_w = np.asarray(gate_w, np.float32)
    w1 = np.asarray(w1, np.float32)
    w2 = np.asarray(w2, np.float32)
    w3 = np.asarray(w3, np.float32)
    sw1 = np.asarray(sw1, np.float32)
    sw2 = np.asarray(sw2, np.float32)
    sw3 = np.asarray(sw3, np.float32)

    in_maps = []
    for c in range(N_CORES):
        mine = [EPC * c + j for j in range(EPC)]
        perm = mine + [e for e in range(E) if e not in mine]
        gwt = np.ascontiguousarray(gate_w[perm].T)        # [D, E] fp32
        gh = gwt.astype(BF)
        gl = (gwt - gh.astype(np.float32)).astype(BF)
        w1t = np.stack([np.ascontiguousarray(w1[e].T) for e in mine]).astype(BF)
        w3t = np.stack([np.ascontiguousarray(w3[e].T) for e in mine]).astype(BF)
        w2t = np.stack([np.ascontiguousarray(w2[e].T) for e in mine]).astype(BF)
        ssl = slice(SIC * c, SIC * (c + 1))
        sw1t = np.ascontiguousarray(sw1[ssl, :].T).astype(BF)  # [D, SIC]
        sw3t = np.ascontiguousarray(sw3[ssl, :].T).astype(BF)
        sw2t = np.ascontiguousarray(sw2[:, ssl].T).astype(BF)  # [SIC, D]
        in_maps.append({
            "xtb": xtb, "xtl": xtl, "gh": gh, "gl": gl,
            "w1t": w1t, "w3t": w3t, "w2t": w2t,
            "sw1t": sw1t, "sw3t": sw3t, "sw2t": sw2t,
        })

    if "nc" not in _cached:
        _cached["nc"] = _build()
    nc = _cached["nc"]

    res = run_bass_kernel_spmd(nc, in_maps, core_ids=list(range(N_CORES)))
    last_exec_time_ns = res.exec_time_ns

    yt = np.concatenate([res.results[c]["out"].astype(np.float32)
                         for c in range(N_CORES)], axis=0)  # [D, T]
    return np.ascontiguousarray(yt.T).reshape(B, S, DIM).astype(np.float32)
